# revision 1
# baseline (speedup 1.0000x reference)
"""DecorrelateBN Trainium2 kernel (8-core SPMD, raw Bass).

Math (matches reference):
  x0 = x * mask                                   (mask zeroes dropped points)
  grouped view: group g = channels [8g, 8g+8); j = channel offset in group
  Gram65 = sum over (t, j) of [x0_j; mask]^T [x0_j; mask]   (X_j[t,g] = x0[t,8g+j])
  -> all-reduced over 8 cores.  s = Gram65[64,:64], Gn = Gram65[64,64]
  cov = (Gram0 - s (s/Gn)^T)/Gn + eps I ; deconv = NewtonSchulz_isqrt(cov, 5)
  out[t, 8g+j] = mask[t] * w[8g+j] * (sum_g' deconv[g,g'] x[t,8g'+j] - dm[g]) + b[8g+j]
  where dm = deconv @ (s/Gn).

Per core: 4 batches = 16384 points.  32 supertiles (ST) of 512 points.
Phase 1 streams x and accumulates Gram65 in PSUM (bf16 matmuls).
Phase 2: AllReduce (17 KB) + Newton-Schulz on-device (fp32).
Phase 3 streams x again: ACT permutes channels j-major, PE transposes to
channel-major, applies block-diag deconv (fp32), transposes back, DVE applies
mask+bias with the strided channel scatter, DMA out.
"""
import sys

sys.path.insert(0, "/opt/trn_rl_repo")

import numpy as np
import ml_dtypes
import concourse.bass as bass
from concourse import mybir
from concourse.bass_utils import run_bass_kernel_spmd

F32 = mybir.dt.float32
BF16 = mybir.dt.bfloat16

N_CORES = 8
B, N, C = 32, 4096, 512
G = 64
J = C // G              # 8
PTS = (B // N_CORES) * N            # 16384 points per core
NST = PTS // 512                    # 32 supertiles
NT = PTS // 128                     # 128 tiles
EPS = 1e-4
N_ITER = 5

_cache = {}


def build(n_cores, use_collective=True):
    nc = bass.Bass(target_bir_lowering=False)

    x = nc.declare_dram_parameter("x", [PTS, C], F32, isOutput=False)
    maskt = nc.declare_dram_parameter("maskt", [128, NT], F32, isOutput=False)
    wperm = nc.declare_dram_parameter("wperm", [128, 4], F32, isOutput=False)
    biasb = nc.declare_dram_parameter("biasb", [128, C], F32, isOutput=False)
    eye = nc.declare_dram_parameter("eye", [128, 128], F32, isOutput=False)
    epseye = nc.declare_dram_parameter("epseye", [64, 64], F32, isOutput=False)
    i15 = nc.declare_dram_parameter("i15", [64, 64], F32, isOutput=False)
    ones8 = nc.declare_dram_parameter("ones8", [128, 8], F32, isOutput=False)
    onesrow = nc.declare_dram_parameter("onesrow", [1, 512], BF16, isOutput=False)
    ones64 = nc.declare_dram_parameter("ones64", [64, 1], F32, isOutput=False)
    ones65 = nc.declare_dram_parameter("ones65", [65, 64], F32, isOutput=False)
    dzero = nc.declare_dram_parameter("dzero", [128, 128], F32, isOutput=False)
    out = nc.declare_dram_parameter("out", [PTS, C], F32, isOutput=True)

    cc_in = nc.dram_tensor("cc_in", [65, 65], F32)
    cc_out = nc.dram_tensor("cc_out", [65, 65], F32, addr_space="Shared")

    CONSTS = [("maskt_s", maskt, [128, NT]), ("wperm_s", wperm, [128, 4]),
              ("biasb_s", biasb, [128, C]), ("eye_s", eye, [128, 128]),
              ("epseye_s", epseye, [64, 64]), ("i15_s", i15, [64, 64]),
              ("ones8_s", ones8, [128, 8]), ("ones64_s", ones64, [64, 1]),
              ("ones65_s", ones65, [65, 64])]
    NCONST = len(CONSTS) + 2  # + onesrow (bf16) + dzero->deconv2

    import contextlib
    ctx = contextlib.ExitStack()
    sb = {}
    for nm, _, shp in CONSTS:
        sb[nm] = ctx.enter_context(nc.sbuf_tensor(nm, shp, F32))
    onesrow_s = ctx.enter_context(nc.sbuf_tensor("onesrow_s", [1, 512], BF16))
    deconv2 = ctx.enter_context(nc.sbuf_tensor("deconv2", [128, 128], F32))

    xst = [ctx.enter_context(nc.sbuf_tensor(f"xst{i}", [128, 4, 512], F32))
           for i in range(2)]
    xm = [ctx.enter_context(nc.sbuf_tensor(f"xm{i}", [128, 520], BF16))
          for i in range(2)]
    xp = [ctx.enter_context(nc.sbuf_tensor(f"xp{i}", [128, 4, 512], F32))
          for i in range(2)]
    xt1 = [ctx.enter_context(nc.sbuf_tensor(f"xt1_{i}", [128, 4, 512], F32))
           for i in range(2)]
    asb = [ctx.enter_context(nc.sbuf_tensor(f"asb{i}", [128, 4, 512], F32))
           for i in range(2)]
    osb = [ctx.enter_context(nc.sbuf_tensor(f"osb{i}", [128, 4, 512], F32))
           for i in range(2)]

    gram_sb = ctx.enter_context(nc.sbuf_tensor("gram_sb", [65, 65], F32))
    g2 = ctx.enter_context(nc.sbuf_tensor("g2", [65, 66], F32))
    # phase-2 small tiles
    lane64 = ctx.enter_context(nc.sbuf_tensor("lane64", [65, 66], F32))  # rows 64: invgn, meanrow
    covu = ctx.enter_context(nc.sbuf_tensor("covu", [64, 64], F32))
    cov = ctx.enter_context(nc.sbuf_tensor("cov", [64, 64], F32))
    scr64 = ctx.enter_context(nc.sbuf_tensor("scr64", [64, 64], F32))
    rowsq = ctx.enter_context(nc.sbuf_tensor("rowsq", [64, 1], F32))
    smalls = ctx.enter_context(nc.sbuf_tensor("smalls", [64, 8], F32))
    # smalls cols: 0 invcol, 1 ninvcol, 2 rcol, 3 mcol; row0 of col 4: nfro, 5: ninv, 6: sq2, 7: rinv
    Yt = ctx.enter_context(nc.sbuf_tensor("Yt", [64, 64], F32))
    Zt = ctx.enter_context(nc.sbuf_tensor("Zt", [64, 64], F32))
    Tt = ctx.enter_context(nc.sbuf_tensor("Tt", [64, 64], F32))
    dcv = ctx.enter_context(nc.sbuf_tensor("dcv", [64, 64], F32))
    negdm2 = ctx.enter_context(nc.sbuf_tensor("negdm2", [1, 128], BF16))

    ps = [ctx.enter_context(nc.psum_tensor(f"ps{i}", [128, 512], F32))
          for i in range(8)]
    # ps6: gram [0:65,0:65] (phase1) + NS ZY/Znew; ps7: NS scratch
    # ps0/1: pxt1 slots; ps2/3: pa slots; ps4/5: pto slots

    sems = {}
    for nm in ["sconst", "sx", "sxm", "sgr", "sxrd", "sge", "scol", "sccd",
               "sp2", "stin", "sxt1e", "sap", "sasbe", "sto", "se2", "sod"]:
        sems[nm] = ctx.enter_context(nc.semaphore(nm))
    sconst, sx, sxm, sgr, sxrd, sge, scol, sccd, sp2, stin, sxt1e, sap, \
        sasbe, sto, se2, sod = (sems[k] for k in
        ["sconst", "sx", "sxm", "sgr", "sxrd", "sge", "scol", "sccd", "sp2",
         "stin", "sxt1e", "sap", "sasbe", "sto", "se2", "sod"])

    x_r = x[:].rearrange("(s k p) c -> s p k c", p=128, k=4)
    out_r = out[:].rearrange("(s k p) c -> s p k c", p=128, k=4)

    # ---------------- phase 2 step list ----------------
    # each: (engine_name, fn(eng)) ; fn emits exactly ONE instruction and
    # returns it (for then_inc).
    A = mybir.AluOpType
    ACT_F = mybir.ActivationFunctionType
    p2_steps = []

    def step(eng):
        def deco(fn):
            p2_steps.append((eng, fn))
            return fn
        return deco

    invgn = lane64[64:65, 0:1]
    meanrow = lane64[64:65, 1:65]
    invcol = smalls[0:64, 0:1]
    ninvcol = smalls[0:64, 1:2]
    rcol = smalls[0:64, 2:3]
    mcol = smalls[0:64, 3:4]
    nfro = smalls[0:1, 4:5]
    ninv = smalls[0:1, 5:6]
    sq2 = smalls[0:1, 6:7]
    rinv = smalls[0:1, 7:8]

    step("vector")(lambda e: e.reciprocal(invgn, g2[64:65, 64:65]))
    step("vector")(lambda e: e.tensor_scalar_mul(meanrow, g2[64:65, 0:64], invgn))
    step("tensor")(lambda e: e.matmul(ps[7][0:64, 0:64], g2[64:65, 0:64],
                                      meanrow, start=True, stop=True))
    step("tensor")(lambda e: e.matmul(ps[7][0:64, 64:65], sb["ones65_s"][64:65, :],
                                      invgn, start=True, stop=True))
    step("vector")(lambda e: e.tensor_copy(invcol, ps[7][0:64, 64:65]))
    step("vector")(lambda e: e.tensor_tensor(covu[:], g2[0:64, 0:64],
                                             ps[7][0:64, 0:64], A.subtract))
    step("vector")(lambda e: e.scalar_tensor_tensor(
        out=cov[:], in0=covu[:], scalar=invcol, in1=sb["epseye_s"][:],
        op0=A.mult, op1=A.add))
    step("vector")(lambda e: e.tensor_tensor(scr64[:], cov[:], cov[:], A.mult))
    step("vector")(lambda e: e.tensor_reduce(rowsq[:], scr64[:],
                                             mybir.AxisListType.X, A.add))
    step("tensor")(lambda e: e.matmul(ps[7][0:1, 65:66], rowsq[:],
                                      sb["ones64_s"][:], start=True, stop=True))
    step("scalar")(lambda e: e.activation(nfro, ps[7][0:1, 65:66], ACT_F.Sqrt))
    step("vector")(lambda e: e.reciprocal(ninv, nfro))
    step("tensor")(lambda e: e.matmul(ps[7][0:64, 66:67], sb["ones65_s"][0:1, :],
                                      ninv, start=True, stop=True))
    step("vector")(lambda e: e.tensor_copy(ninvcol, ps[7][0:64, 66:67]))
    step("vector")(lambda e: e.tensor_scalar_mul(Yt[:], cov[:], ninvcol))
    step("vector")(lambda e: e.tensor_copy(Zt[:], sb["eye_s"][0:64, 0:64]))
    for _ in range(N_ITER):
        step("tensor")(lambda e: e.matmul(ps[6][0:64, 0:64], Zt[:], Yt[:],
                                          start=True, stop=True))
        step("vector")(lambda e: e.scalar_tensor_tensor(
            out=Tt[:], in0=ps[6][0:64, 0:64], scalar=-0.5, in1=sb["i15_s"][:],
            op0=A.mult, op1=A.add))
        step("tensor")(lambda e: e.matmul(ps[7][0:64, 0:64], Yt[:], Tt[:],
                                          start=True, stop=True))
        step("tensor")(lambda e: e.matmul(ps[6][0:64, 64:128], Tt[:], Zt[:],
                                          start=True, stop=True))
        step("vector")(lambda e: e.tensor_copy(Yt[:], ps[7][0:64, 0:64]))
        step("vector")(lambda e: e.tensor_copy(Zt[:], ps[6][0:64, 64:128]))
    step("scalar")(lambda e: e.activation(sq2, nfro, ACT_F.Sqrt))
    step("vector")(lambda e: e.reciprocal(rinv, sq2))
    step("tensor")(lambda e: e.matmul(ps[7][0:64, 67:68], sb["ones65_s"][0:1, :],
                                      rinv, start=True, stop=True))
    step("vector")(lambda e: e.tensor_copy(rcol, ps[7][0:64, 67:68]))
    step("vector")(lambda e: e.tensor_scalar_mul(dcv[:], Zt[:], rcol))
    step("vector")(lambda e: e.tensor_copy(deconv2[0:64, 0:64], dcv[:]))
    step("vector")(lambda e: e.tensor_copy(deconv2[64:128, 64:128], dcv[:]))
    step("vector")(lambda e: e.tensor_tensor(mcol, g2[0:64, 64:65], invcol,
                                             A.mult))
    step("tensor")(lambda e: e.matmul(ps[7][0:1, 128:192], mcol, dcv[:],
                                      start=True, stop=True))
    step("scalar")(lambda e: e.activation(negdm2[0:1, 0:64], ps[7][0:1, 128:192],
                                          ACT_F.Copy, bias=0.0, scale=-1.0))
    step("scalar")(lambda e: e.activation(negdm2[0:1, 64:128], ps[7][0:1, 128:192],
                                          ACT_F.Copy, bias=0.0, scale=-1.0))
    NSDONE = len(p2_steps)

    def emit_p2(eng_name, eng):
        for t, (enm, fn) in enumerate(p2_steps):
            if enm != eng_name:
                continue
            if t == 0:
                eng.wait_ge(scol, 32)
            else:
                eng.wait_ge(sp2, t)
            fn(eng).then_inc(sp2, 1)

    with nc.Block() as block:

        @block.sync
        def _(sync):
            for nm, src, shp in CONSTS:
                sync.dma_start(out=sb[nm][:], in_=src[:]).then_inc(sconst, 16)
            sync.dma_start(out=onesrow_s[:], in_=onesrow[:]).then_inc(sconst, 16)
            sync.dma_start(out=deconv2[:], in_=dzero[:]).then_inc(sconst, 16)
            # phase 1 + phase 3 supertile loads
            for sg in range(2 * NST):
                if sg >= 2:
                    sync.wait_ge(sxrd, sg - 1)
                sync.dma_start(out=xst[sg % 2][:],
                               in_=x_r[sg % NST]).then_inc(sx, 16)

        @block.scalar
        def _(scalar):
            scalar.wait_ge(sconst, 16 * NCONST)
            # ---- phase 1: masked bf16 tiles ----
            for i in range(NT):
                s, k = i // 4, i % 4
                if k == 0:
                    scalar.wait_ge(sx, 16 * (s + 1))
                if i >= 2:
                    scalar.wait_ge(sgr, i - 1)
                mcolv = sb["maskt_s"][:, i:i + 1]
                scalar.activation(xm[i % 2][:, 0:512], xst[s % 2][:, k, :],
                                  ACT_F.Copy, bias=0.0, scale=mcolv)
                ins = scalar.activation(xm[i % 2][:, 512:520], sb["ones8_s"][:],
                                        ACT_F.Copy, bias=0.0, scale=mcolv)
                ins.then_inc(sxm, 1)
                if k == 3:
                    scalar.nop().then_inc(sxrd, 1)
            # ---- phase 2 ----
            emit_p2("scalar", scalar)
            # ---- phase 3: permutes + evacs ----
            for s in range(NST):
                scalar.wait_ge(sx, 16 * (NST + s + 1))
                if s >= 2:
                    scalar.wait_ge(stin, 4 * s - 4)
                for k in range(4):
                    ins = scalar.copy(
                        xp[s % 2][:, k, :].rearrange("p (j g) -> p j g", j=8),
                        xst[s % 2][:, k, :].rearrange("p (g j) -> p j g", j=8))
                ins.then_inc(sxrd, 1)
                for a in range(4):
                    e = 4 * s + a
                    # evac xT1 psum -> sbuf
                    scalar.wait_ge(stin, e + 1)
                    scalar.copy(xt1[s % 2][:, a, :],
                                ps[e % 2][:, 0:512]).then_inc(sxt1e, 1)
                    # evac apply psum -> sbuf (weighted)
                    scalar.wait_ge(sap, e + 1)
                    scalar.activation(asb[s % 2][:, a, :], ps[2 + e % 2][:, 0:512],
                                      ACT_F.Copy, bias=0.0,
                                      scale=sb["wperm_s"][:, a:a + 1]
                                      ).then_inc(sasbe, 1)

        @block.tensor
        def _(tensor):
            # ---- phase 1: Gram65 ----
            for i in range(NT):
                tensor.wait_ge(sxm, i + 1)
                xv = xm[i % 2][:].rearrange("p (g j) -> p j g", j=8)
                for j in range(8):
                    mm = tensor.matmul(ps[6][0:65, 0:65], xv[:, j, :], xv[:, j, :],
                                       start=(i == 0 and j == 0),
                                       stop=(i == NT - 1 and j == 7))
                mm.then_inc(sgr, 1)
            # ---- phase 2 ----
            tensor.wait_ge(sconst, 16 * NCONST)
            emit_p2("tensor", tensor)
            # ---- phase 3 ----
            for s in range(NST):

                def t_in(a):
                    e = 4 * s + a
                    if a == 0:
                        tensor.wait_ge(sxrd, NST + s + 1)
                    if e >= 2:
                        tensor.wait_ge(sxt1e, e - 1)
                    for k in range(4):
                        mm = tensor.transpose(ps[e % 2][:, k * 128:(k + 1) * 128],
                                              xp[s % 2][:, k, a * 128:(a + 1) * 128],
                                              sb["eye_s"][:])
                    mm.then_inc(stin, 1)

                def t_apply(a):
                    e = 4 * s + a
                    tensor.wait_ge(sxt1e, e + 1)
                    if e >= 2:
                        tensor.wait_ge(sasbe, e - 1)
                    tensor.matmul(ps[2 + e % 2][:, 0:512], deconv2[:],
                                  xt1[s % 2][:, a, :], start=True, stop=False)
                    tensor.matmul(ps[2 + e % 2][:, 0:512], negdm2[:],
                                  onesrow_s[:], start=False,
                                  stop=True).then_inc(sap, 1)

                t_in(0)
                if s == 0:
                    tensor.wait_ge(sp2, NSDONE)
                t_in(1)
                t_apply(0)
                t_in(2)
                t_apply(1)
                t_in(3)
                t_apply(2)
                t_apply(3)
                for k in range(4):
                    e = 4 * s + k
                    if k == 0:
                        tensor.wait_ge(sasbe, 4 * s + 4)
                    if e >= 2:
                        tensor.wait_ge(se2, e - 1)
                    for a in range(4):
                        mm = tensor.transpose(ps[4 + e % 2][:, a * 128:(a + 1) * 128],
                                              asb[s % 2][:, a, k * 128:(k + 1) * 128],
                                              sb["eye_s"][:])
                    mm.then_inc(sto, 1)

        @block.vector
        def _(vector):
            vector.wait_ge(sconst, 16 * NCONST)
            # ---- phase 1: gram evacuation ----
            vector.wait_ge(sgr, NT)
            vector.tensor_copy(gram_sb[:], ps[6][0:65, 0:65]).then_inc(sge, 1)
            # ---- phase 2 ----
            emit_p2("vector", vector)
            # ---- phase 3: evac2 ----
            for s in range(NST):
                for k in range(4):
                    e = 4 * s + k
                    vector.wait_ge(sto, e + 1)
                    if s >= 2:
                        vector.wait_ge(sod, 16 * (s - 1))
                    i = 4 * s + k
                    vector.scalar_tensor_tensor(
                        out=osb[s % 2][:, k, :].rearrange(
                            "p (g a j) -> p a j g", a=4, j=2),
                        in0=ps[4 + e % 2][:, 0:512].rearrange(
                            "p (a j g) -> p a j g", a=4, j=2),
                        scalar=sb["maskt_s"][:, i:i + 1],
                        in1=sb["biasb_s"][:].rearrange(
                            "p (g a j) -> p a j g", a=4, j=2),
                        op0=A.mult, op1=A.add).then_inc(se2, 1)

        @block.gpsimd
        def _(gpsimd):
            gpsimd.wait_ge(sge, 1)
            gpsimd.dma_start(out=cc_in[:], in_=gram_sb[:]).then_inc(scol, 16)
            gpsimd.wait_ge(scol, 16)
            if use_collective:
                gpsimd.collective_compute(
                    "AllReduce", A.add,
                    replica_groups=[list(range(n_cores))],
                    ins=[cc_in[:]],
                    outs=[cc_out[:]]).then_inc(sccd, 1)
                gpsimd.wait_ge(sccd, 1)
                gpsimd.dma_start(out=g2[0:65, 0:65], in_=cc_out[:]).then_inc(scol, 16)
            else:
                gpsimd.dma_start(out=g2[0:65, 0:65], in_=cc_in[:]).then_inc(scol, 16)
            # phase 3 stores
            for s in range(NST):
                gpsimd.wait_ge(se2, 4 * s + 4)
                gpsimd.dma_start(out=out_r[s],
                                 in_=osb[s % 2][:]).then_inc(sod, 16)
            gpsimd.wait_ge(sod, 16 * NST)

    ctx.close()
    return nc


def _host_aux():
    w = _cache["weight"]
    b = _cache["bias"]
    w2 = w.reshape(64, 8)
    wperm = np.empty((128, 4), dtype=np.float32)
    for a in range(4):
        for jj in range(2):
            wperm[jj * 64:(jj + 1) * 64, a] = w2[:, 2 * a + jj]
    biasb = np.tile(b.astype(np.float32)[None, :], (128, 1))
    aux = {
        "wperm": wperm,
        "biasb": np.ascontiguousarray(biasb),
        "eye": np.eye(128, dtype=np.float32),
        "epseye": (EPS * np.eye(64)).astype(np.float32),
        "i15": (1.5 * np.eye(64)).astype(np.float32),
        "ones8": np.ones((128, 8), dtype=np.float32),
        "onesrow": np.ones((1, 512), dtype=ml_dtypes.bfloat16),
        "ones64": np.ones((64, 1), dtype=np.float32),
        "ones65": np.ones((65, 64), dtype=np.float32),
        "dzero": np.zeros((128, 128), dtype=np.float32),
    }
    return aux


def kernel(coords, x, mask, weight, bias, _trace=False):
    x = np.asarray(x, dtype=np.float32)
    mask = np.asarray(mask)
    weight = np.asarray(weight, dtype=np.float32)
    bias = np.asarray(bias, dtype=np.float32)
    _cache["weight"] = weight
    _cache["bias"] = bias
    aux = _host_aux()

    if "nc" not in _cache:
        _cache["nc"] = build(N_CORES)
    nc = _cache["nc"]

    in_maps = []
    bpc = B // N_CORES
    for c in range(N_CORES):
        xc = np.ascontiguousarray(
            x[c * bpc:(c + 1) * bpc].reshape(PTS, C))
        m = mask[c * bpc:(c + 1) * bpc].reshape(PTS).astype(np.float32)
        maskt = np.ascontiguousarray(m.reshape(NT, 128).T)
        im = {"x": xc, "maskt": maskt}
        im.update(aux)
        in_maps.append(im)

    res = run_bass_kernel_spmd(nc, in_maps, core_ids=list(range(N_CORES)),
                               trace=_trace)
    outs = [res.results[c]["out"].reshape(bpc, N, C) for c in range(N_CORES)]
    full = np.concatenate(outs, axis=0)
    if _trace:
        return full, res
    return full

